# revision 2
# baseline (speedup 1.0000x reference)
"""WaveNet dilated-conv stack (30 layers) on 8 TRN2 NeuronCores.

Data-parallel over batch: core c handles batches [2c, 2c+1].
Per core, two streams: stream A = local batch 0 (SBUF partitions 0-63,
row-group-lo matmuls), stream B = local batch 1 (H state at partitions
64-127, row-group-hi matmuls). Per-stream PSUM banks so every bank is
written by a single PE row group (cross-group banks fault the exec unit).

Per layer & 512-col time tile (per stream):
  conv  : 2 fp32r matmuls (taps) -> E bank [128,512] = [a-half; sigma-half]
  act   : Tanh(a)+bias -> T (bf16, at stream's partitions, cross-partition
          ACT write), Sigmoid(s)+bias -> S
  gate  : z = T*S, one bf16 DVE mul (same-base operands)
  res   : bf16 matmul -> R bank rows 0-63; fused DVE scalar_tensor_tensor
          x' = (R + res_b) + x  (PSUM base-0 + SBUF base-p0 mixed is legal)
  skip  : 2 bf16 matmuls accumulating into S1/S2 banks across all 30 layers
History: per-layer SBUF windows [128, d+512] with DMA tail-shift for d<512;
for d=512 a 2-slot ring (no copies). Skip biases summed once into the
PSUM-eviction activation.
"""

import numpy as np

NR_LAYERS = 10
STACK_TIME = 3
C = 64
S = 256
B = 16
T = 4096
L = NR_LAYERS * STACK_TIME
DILATIONS = [2 ** (i % NR_LAYERS) for i in range(L)]
NCORES = 8
BPC = B // NCORES          # batches per core
NT = 512                   # time-tile columns
NTILES = T // NT

_CACHE = {}


def _round_f32r(a):
    """Round fp32 array to the fp32r grid (11-bit mantissa, round-half-even-ish)."""
    a = np.ascontiguousarray(a, dtype=np.float32)
    u = a.view(np.uint32)
    r = (u + 0x7FF + ((u >> 12) & 1)) & np.uint32(0xFFFFF000)
    return r.view(np.float32).copy()


def _build():
    import concourse.bacc as bacc
    import concourse.mybir as mybir
    import concourse.tile as tile

    F32 = mybir.dt.float32
    F32R = mybir.dt.float32r
    BF16 = mybir.dt.bfloat16
    ALU = mybir.AluOpType
    AF = mybir.ActivationFunctionType

    nc = bacc.Bacc("TRN2", target_bir_lowering=False, debug=False,
                   num_devices=NCORES)

    fwd = nc.dram_tensor("fwd", [BPC, C, T], F32R, kind="ExternalInput").ap()
    wc_d = nc.dram_tensor("convw", [128, L * 256], F32R, kind="ExternalInput").ap()
    wr_d = nc.dram_tensor("resw", [128, L * 64], BF16, kind="ExternalInput").ap()
    wk_d = nc.dram_tensor("skipw", [128, L * 256], BF16, kind="ExternalInput").ap()
    ab_d = nc.dram_tensor("actbias", [128, L], F32, kind="ExternalInput").ap()
    zz_d = nc.dram_tensor("zeros", [128, NT], F32R, kind="ExternalInput").ap()
    rb_d = nc.dram_tensor("rbias", [128, L], F32, kind="ExternalInput").ap()
    sb_d = nc.dram_tensor("sbias", [128, 2], F32, kind="ExternalInput").ap()
    out_d = nc.dram_tensor("out", [BPC, S, T], F32, kind="ExternalOutput").ap()

    with tile.TileContext(nc) as tc, \
         tc.tile_pool(name="wpool", bufs=1) as wpool, \
         tc.tile_pool(name="hpool", bufs=1) as hpool, \
         tc.tile_pool(name="work", bufs=3) as work, \
         tc.tile_pool(name="stage", bufs=3) as stage, \
         tc.tile_pool(name="pp", bufs=1, space="PSUM") as pp:

        # ---- weights ----
        wc = wpool.tile([128, L * 256], F32R, name="wc")
        wr = wpool.tile([128, L * 64], BF16, name="wr")
        wk = wpool.tile([128, L * 256], BF16, name="wk")
        ab = wpool.tile([128, L], F32, name="ab")
        rb = wpool.tile([128, L], F32, name="rb")
        sb2 = wpool.tile([128, 2], F32, name="sb2")
        for dst, src in ((wc, wc_d), (wr, wr_d), (wk, wk_d), (ab, ab_d),
                         (rb, rb_d), (sb2, sb_d)):
            nc.sync.dma_start(dst[:], src[:])

        # ---- history buffers (layers 1..L-1) ----
        H = {}
        for i in range(1, L):
            d = DILATIONS[i]
            if d < NT:
                H[i] = hpool.tile([128, d + NT], F32R, name=f"h{i}")
                nc.sync.dma_start(H[i][:, 0:d], zz_d[:, 0:d])
            else:  # d == NT: 2-slot ring
                H[i] = hpool.tile([128, 2 * NT], F32R, name=f"h{i}")
                nc.sync.dma_start(H[i][:, NT:2 * NT], zz_d[:, :])  # slot1 = tile -1

        # ---- PSUM banks: per stream E, R, S1, S2 ----
        E = [pp.tile([128, NT], F32, name=f"E{s}") for s in range(2)]
        R = [pp.tile([128, NT], F32, name=f"R{s}") for s in range(2)]
        SK = [[pp.tile([128, NT], F32, name=f"SK{s}_{cch}") for cch in range(2)]
              for s in range(2)]

        for k in range(NTILES):
            # layer-0 input windows (1 col of history for d=1)
            h0 = work.tile([128, NT + 1], F32R, name="h0", tag="h0")
            for s in range(2):
                p0 = 64 * s
                if k == 0:
                    nc.sync.dma_start(h0[p0:p0 + 64, 0:1], zz_d[p0:p0 + 64, 0:1])
                    nc.sync.dma_start(h0[p0:p0 + 64, 1:NT + 1],
                                      fwd[s, :, 0:NT])
                else:
                    nc.sync.dma_start(h0[p0:p0 + 64, :],
                                      fwd[s, :, k * NT - 1:(k + 1) * NT])

            for i in range(L):
                d = DILATIONS[i]
                # tap APs for layer input x_i at this tile
                if i == 0:
                    tap0 = h0[:, 0:NT]
                    tap1 = h0[:, 1:NT + 1]
                elif d < NT:
                    tap0 = H[i][:, 0:NT]
                    tap1 = H[i][:, d:d + NT]
                else:
                    cur = (k % 2) * NT
                    prev = ((k + 1) % 2) * NT
                    tap0 = H[i][:, prev:prev + NT]
                    tap1 = H[i][:, cur:cur + NT]

                for s in range(2):
                    p0 = 64 * s
                    Es, Rs = E[s], R[s]
                    # conv: 2 taps accumulate, fp32r
                    nc.tensor.matmul(Es[:, :], wc[p0:p0 + 64, i * 256:i * 256 + 128],
                                     tap0[p0:p0 + 64, :], start=True, stop=False,
                                     tile_position=(p0, 0), skip_group_check=True)
                    nc.tensor.matmul(Es[:, :], wc[p0:p0 + 64, i * 256 + 128:i * 256 + 256],
                                     tap1[p0:p0 + 64, :], start=False, stop=True,
                                     tile_position=(p0, 0), skip_group_check=True)
                    # activations -> stream's partitions
                    Tt = work.tile([128, NT], BF16, name=f"tt{s}", tag=f"tt{s}")
                    Ss = work.tile([128, NT], BF16, name=f"ss{s}", tag=f"ss{s}")
                    nc.scalar.activation(Tt[p0:p0 + 64, :], Es[0:64, :], AF.Tanh,
                                         bias=ab[0:64, i:i + 1])
                    nc.scalar.activation(Ss[p0:p0 + 64, :], Es[64:128, :], AF.Sigmoid,
                                         bias=ab[64:128, i:i + 1])
                    # gate
                    Z = work.tile([128, NT], BF16, name=f"z{s}", tag=f"z{s}")
                    nc.vector.tensor_tensor(Z[p0:p0 + 64, :], Tt[p0:p0 + 64, :],
                                            Ss[p0:p0 + 64, :], ALU.mult)
                    # skip accumulation (all 30 layers into same banks)
                    for cch in range(2):
                        nc.tensor.matmul(SK[s][cch][:, :],
                                         wk[p0:p0 + 64,
                                            i * 256 + cch * 128:i * 256 + (cch + 1) * 128],
                                         Z[p0:p0 + 64, :],
                                         start=(i == 0), stop=(i == L - 1),
                                         tile_position=(p0, 0), skip_group_check=True)
                    if i == L - 1:
                        continue  # x_30 is never used
                    # residual matmul + fused add
                    nc.tensor.matmul(Rs[0:64, :], wr[p0:p0 + 64, i * 64:(i + 1) * 64],
                                     Z[p0:p0 + 64, :], start=True, stop=True,
                                     tile_position=(p0, 0), skip_group_check=True)
                    dn = DILATIONS[i + 1]
                    if dn < NT:
                        dst = H[i + 1][p0:p0 + 64, dn:dn + NT]
                    else:
                        dst = H[i + 1][p0:p0 + 64, (k % 2) * NT:(k % 2) * NT + NT]
                    nc.vector.scalar_tensor_tensor(dst, Rs[0:64, :],
                                                   rb[p0:p0 + 64, i:i + 1],
                                                   tap1[p0:p0 + 64, :],
                                                   ALU.add, ALU.add)

                # history tail shift for next tile (after conv reads)
                if i >= 1 and d < NT and k < NTILES - 1:
                    nc.sync.dma_start(H[i][:, 0:d], H[i][:, NT:NT + d])

            # evict skip accumulators
            for s in range(2):
                for cch in range(2):
                    ES = stage.tile([128, NT], F32, name=f"es{s}{cch}",
                                    tag=f"es{s}{cch}")
                    nc.scalar.activation(ES[:, :], SK[s][cch][:, :],
                                         AF.Identity, bias=sb2[:, cch:cch + 1])
                    nc.sync.dma_start(
                        out_d[s, cch * 128:(cch + 1) * 128, k * NT:(k + 1) * NT],
                        ES[:, :])
    nc.compile()
    return nc


def _preprocess(dil_w, dil_b, res_w, res_b, skip_w, skip_b):
    import ml_dtypes
    convw = np.zeros((128, L * 256), np.float32)
    resw = np.zeros((128, L * 64), np.float32)
    skipw = np.zeros((128, L * 256), np.float32)
    actbias = np.zeros((128, L), np.float32)
    rbias = np.zeros((128, L), np.float32)
    for i in range(L):
        for tap in range(2):
            lt = dil_w[i, :, :, tap].T  # [64, 128]
            convw[0:64, i * 256 + tap * 128:i * 256 + (tap + 1) * 128] = lt
            convw[64:128, i * 256 + tap * 128:i * 256 + (tap + 1) * 128] = lt
        rt = res_w[i].T                # [64, 64]
        resw[0:64, i * 64:(i + 1) * 64] = rt
        resw[64:128, i * 64:(i + 1) * 64] = rt
        kt = skip_w[i].T               # [64, 256]
        skipw[0:64, i * 256:(i + 1) * 256] = kt
        skipw[64:128, i * 256:(i + 1) * 256] = kt
        actbias[0:64, i] = dil_b[i][0:64]
        actbias[64:128, i] = dil_b[i][64:128]
        rbias[0:64, i] = res_b[i]
        rbias[64:128, i] = res_b[i]
    sbias = np.zeros((128, 2), np.float32)
    sbsum = skip_b.sum(axis=0)
    sbias[:, 0] = sbsum[0:128]
    sbias[:, 1] = sbsum[128:256]
    bf = ml_dtypes.bfloat16
    return {
        "convw": _round_f32r(convw),
        "resw": resw.astype(bf),
        "skipw": skipw.astype(bf),
        "actbias": actbias,
        "rbias": rbias,
        "sbias": sbias,
    }


def kernel(forward_input, dil_w, dil_b, res_w, res_b, skip_w, skip_b,
           _trace=False):
    from concourse import bass_utils

    if "nc" not in _CACHE:
        _CACHE["nc"] = _build()
    nc = _CACHE["nc"]

    shared = _preprocess(np.asarray(dil_w, np.float32), np.asarray(dil_b, np.float32),
                         np.asarray(res_w, np.float32), np.asarray(res_b, np.float32),
                         np.asarray(skip_w, np.float32), np.asarray(skip_b, np.float32))
    fwd = _round_f32r(np.asarray(forward_input, np.float32))
    in_maps = []
    for c in range(NCORES):
        m = dict(shared)
        m["fwd"] = fwd[c * BPC:(c + 1) * BPC]
        m["zeros"] = np.zeros((128, NT), np.float32)
        in_maps.append(m)

    res = bass_utils.run_bass_kernel_spmd(nc, in_maps,
                                          core_ids=list(range(NCORES)),
                                          trace=_trace)
    out = np.concatenate([res.results[c]["out"] for c in range(NCORES)], axis=0)
    _CACHE["last_result"] = res
    return out
